# revision 7
# baseline (speedup 1.0000x reference)
"""Trainium2 Bass kernel for nn_Discrimitor (embedding_lookup two-tower MLP).

Strategy (8 NeuronCores, data-parallel over the batch):
  - Replicate the 1M x 100 f32 embedding table, host-cast to fp16 and pad
    rows to 128 elements (256B rows) -> per-core HBM gather granularity is
    one 256B row.
  - Each core handles 65536 index pairs. Rows are fetched with SWDGE
    indirect DMA (gather): 4096 rows per call, int32 indices resident in
    SBUF, landing batch-major ([128 partitions, 32 rows x 128 fp16]).
  - Per 512-batch compute tile: 8 PE transposes (fp16, via identity) flip
    a/c rows to embed-major, DVE/ACT copy PSUM->SBUF fp16, DVE forms a*c,
    3 accumulating fp16 matmuls (K=128, N=512) compute hidden @ W1 into
    PSUM [64,512], ACT applies relu+b1 -> fp16, one matmul with W2 gives
    logits [1,512], ACT/DVE adds b2 into an output staging row which is
    DMA'd back 4096 logits at a time.

The kernel() entry takes FULL unsharded inputs and returns the FULL
[524288, 1] f32 output.
"""

import sys

for _p in ("/opt/trn_rl_repo",):
    if _p not in sys.path:
        sys.path.insert(0, _p)

import numpy as np

import concourse.bacc as bacc
import concourse.tile as tile
from concourse import bass, mybir
from concourse.bass_utils import run_bass_kernel_spmd
from concourse.masks import make_identity

# ---- problem constants (hardcoded per contract) ----
DOC_SIZE = 1_000_000
EMBED = 100
DP = 128          # padded row length (fp16 -> 256B rows)
H = 64
BATCH = 524288
CORES = 8
BC = BATCH // CORES          # 65536 batch elements per core

# ---- kernel tunables ----
GROUPS = 16                  # gather groups per core
RPG = BC // GROUPS           # rows gathered per call (a and c separately)
JPG = RPG // 128             # rows per partition per gather call
TILE = 512                   # batch elements per compute tile
TPG = RPG // TILE            # compute tiles per gather group

F16 = mybir.dt.float16
F32 = mybir.dt.float32
I32 = mybir.dt.int32
AF = mybir.ActivationFunctionType


def build_nc(doc_rows=DOC_SIZE, bc=BC, groups=GROUPS, reps=1,
             gather_only=False, compute_only=False):
    """Build the per-core Bass module. Parametrized so tests can build a
    small config for CoreSim; reps>1 wraps the body in a hardware loop for
    dispatch-noise-free timing."""
    rpg = bc // groups
    jpg = rpg // 128
    tpg = rpg // TILE
    assert rpg % TILE == 0 and TILE == 512

    nc = bacc.Bacc("TRN2", target_bir_lowering=False, num_swdge_queues=4)

    tbl = nc.dram_tensor("tbl", [doc_rows, DP], F16, kind="ExternalInput")
    ia = nc.dram_tensor("ia", [128, jpg * groups], I32, kind="ExternalInput")
    ic = nc.dram_tensor("ic", [128, jpg * groups], I32, kind="ExternalInput")
    w1 = nc.dram_tensor("w1", [DP, 3 * H], F16, kind="ExternalInput")
    w2 = nc.dram_tensor("w2", [H, 1], F16, kind="ExternalInput")
    b1 = nc.dram_tensor("b1", [H, 1], F32, kind="ExternalInput")
    b2 = nc.dram_tensor("b2", [1, 1], F32, kind="ExternalInput")
    out = nc.dram_tensor("out", [bc], F32, kind="ExternalOutput")

    with tile.TileContext(nc) as tc:
        with (
            tc.tile_pool(name="singles", bufs=1) as singles,
            tc.tile_pool(name="graw", bufs=3) as graw,
            tc.tile_pool(name="tsb", bufs=4) as tsb,
            tc.tile_pool(name="h1sb", bufs=2) as h1sb,
            tc.tile_pool(name="stage", bufs=2) as stagep,
            tc.tile_pool(name="ps_t", bufs=4, space="PSUM") as ps_t,
            tc.tile_pool(name="ps_h1", bufs=2, space="PSUM") as ps_h1,
            tc.tile_pool(name="ps_lg", bufs=2, space="PSUM") as ps_lg,
        ):
            # constants / weights / indices -> SBUF once
            w1_sb = singles.tile([DP, 3 * H], F16)
            nc.sync.dma_start(out=w1_sb[:], in_=w1[:])
            w2_sb = singles.tile([H, 1], F16)
            nc.sync.dma_start(out=w2_sb[:], in_=w2[:])
            b1_sb = singles.tile([H, 1], F32)
            nc.sync.dma_start(out=b1_sb[:], in_=b1[:])
            b2_sb = singles.tile([1, 1], F32)
            nc.sync.dma_start(out=b2_sb[:], in_=b2[:])
            ident = singles.tile([128, 128], F16)
            make_identity(nc, ident[:])
            ia_sb = singles.tile([128, jpg * groups], I32)
            nc.sync.dma_start(out=ia_sb[:], in_=ia[:])
            ic_sb = singles.tile([128, jpg * groups], I32)
            nc.sync.dma_start(out=ic_sb[:], in_=ic[:])

            static_a = None
            if compute_only:
                static_a = singles.tile([128, rpg], F16)
                nc.vector.memset(static_a[:], 0)
                static_c = singles.tile([128, rpg], F16)
                nc.vector.memset(static_c[:], 0)

            rep_cm = tc.For_i(0, reps) if reps > 1 else None
            if rep_cm is not None:
                rep_cm.__enter__()
            for g in range(groups):
                # HW indirect DMA consumes exactly one offset per partition
                # per call -> gather 128 rows (one [128,128] block) per call.
                _qnames = ["qPoolDynamic", "qPoolDynamic1", "qPoolDynamic2",
                           "qPoolDynamic3"]
                if compute_only:
                    a_raw, c_raw = static_a, static_c
                else:
                    a_raw = graw.tile([128, rpg], F16, tag="araw")
                    for j in range(jpg):
                        inst = nc.gpsimd.indirect_dma_start(
                            out=a_raw[:, j * DP : (j + 1) * DP],
                            out_offset=None,
                            in_=tbl[:],
                            in_offset=bass.IndirectOffsetOnAxis(
                                ap=ia_sb[:, g * jpg + j : g * jpg + j + 1], axis=0
                            ),
                        )
                        inst.ins.queue = _qnames[j % 4]
                    c_raw = graw.tile([128, rpg], F16, tag="craw")
                    for j in range(jpg):
                        inst = nc.gpsimd.indirect_dma_start(
                            out=c_raw[:, j * DP : (j + 1) * DP],
                            out_offset=None,
                            in_=tbl[:],
                            in_offset=bass.IndirectOffsetOnAxis(
                                ap=ic_sb[:, g * jpg + j : g * jpg + j + 1], axis=0
                            ),
                        )
                        inst.ins.queue = _qnames[j % 4]

                if gather_only:
                    # consume the gathered tiles with a cheap DMA so pool
                    # backpressure still applies, skip all compute
                    nc.sync.dma_start(
                        out=out[g * rpg : g * rpg + rpg // 2].rearrange(
                            "(o n) -> o n", o=1),
                        in_=a_raw[0:1, :].bitcast(F32),
                    )
                    nc.sync.dma_start(
                        out=out[g * rpg + rpg // 2 : (g + 1) * rpg].rearrange(
                            "(o n) -> o n", o=1),
                        in_=c_raw[0:1, :].bitcast(F32),
                    )
                    continue

                stage = stagep.tile([1, rpg], F32)

                for tt in range(tpg):
                    aT = tsb.tile([128, TILE], F16, tag="aT")
                    cT = tsb.tile([128, TILE], F16, tag="cT")
                    for u in range(4):
                        k = (tt * 4 + u) * 128
                        tp_a = ps_t.tile([128, 128], F16, tag="pst")
                        nc.tensor.transpose(
                            tp_a[:], a_raw[:, k : k + 128], ident[:]
                        )
                        nc.vector.tensor_copy(
                            out=aT[:, u * 128 : (u + 1) * 128], in_=tp_a[:]
                        )
                        tp_c = ps_t.tile([128, 128], F16, tag="pst")
                        nc.tensor.transpose(
                            tp_c[:], c_raw[:, k : k + 128], ident[:]
                        )
                        nc.scalar.activation(
                            out=cT[:, u * 128 : (u + 1) * 128],
                            in_=tp_c[:],
                            func=AF.Copy,
                        )
                    acT = tsb.tile([128, TILE], F16, tag="acT")
                    nc.vector.tensor_mul(acT[:], aT[:], cT[:])

                    h1p = ps_h1.tile([H, TILE], F32, tag="h1p")
                    nc.tensor.matmul(
                        h1p[:], w1_sb[:, 0:H], aT[:], start=True, stop=False
                    )
                    nc.tensor.matmul(
                        h1p[:], w1_sb[:, H : 2 * H], cT[:], start=False, stop=False
                    )
                    nc.tensor.matmul(
                        h1p[:], w1_sb[:, 2 * H : 3 * H], acT[:],
                        start=False, stop=True,
                    )
                    h1s = h1sb.tile([H, TILE], F16, tag="h1s")
                    nc.scalar.activation(
                        out=h1s[:], in_=h1p[:], func=AF.Relu, bias=b1_sb[:],
                        scale=1.0,
                    )
                    lgp = ps_lg.tile([1, TILE], F32, tag="lgp")
                    nc.tensor.matmul(
                        lgp[:], w2_sb[:], h1s[:], start=True, stop=True
                    )
                    dst = stage[0:1, tt * TILE : (tt + 1) * TILE]
                    if tt % 2 == 0:
                        nc.scalar.activation(
                            out=dst, in_=lgp[:], func=AF.Identity, bias=b2_sb[:],
                            scale=1.0,
                        )
                    else:
                        nc.vector.tensor_add(
                            out=dst, in0=lgp[:],
                            in1=b2_sb[:].to_broadcast([1, TILE]),
                        )

                nc.sync.dma_start(
                    out=out[g * rpg : (g + 1) * rpg].rearrange("(o n) -> o n", o=1),
                    in_=stage[:],
                )
            if rep_cm is not None:
                rep_cm.__exit__(None, None, None)

    nc.compile()
    return nc


def prep_inputs(anchor_h, candidate_h, doc_embed, W1, b1, W2, b2,
                bc=BC, groups=GROUPS, cores=CORES):
    """Host-side packing of full inputs into per-core in_maps."""
    jpg = bc // groups // 128

    tbl16 = np.zeros((doc_embed.shape[0], DP), np.float16)
    tbl16[:, :EMBED] = np.asarray(doc_embed, np.float32)

    # W1 rows: [a(100); c(100); ac(100)] -> padded chunks of 128
    W1 = np.asarray(W1, np.float32)
    w1p = np.zeros((DP, 3 * H), np.float16)
    w1p[:EMBED, 0:H] = W1[0:EMBED]
    w1p[:EMBED, H : 2 * H] = W1[EMBED : 2 * EMBED]
    w1p[:EMBED, 2 * H : 3 * H] = W1[2 * EMBED : 3 * EMBED]

    w2p = np.asarray(W2, np.float32).astype(np.float16).reshape(H, 1)
    b1p = np.asarray(b1, np.float32).reshape(H, 1)
    b2p = np.asarray(b2, np.float32).reshape(1, 1)

    a_all = np.asarray(anchor_h).astype(np.int32)
    c_all = np.asarray(candidate_h).astype(np.int32)

    in_maps = []
    for c in range(cores):
        sl = slice(c * bc, (c + 1) * bc)
        # layout[p, g*jpg + j] = idx[g*rpg + j*128 + p]
        ia = (
            a_all[sl].reshape(groups, jpg, 128).transpose(2, 0, 1)
            .reshape(128, groups * jpg).copy()
        )
        icx = (
            c_all[sl].reshape(groups, jpg, 128).transpose(2, 0, 1)
            .reshape(128, groups * jpg).copy()
        )
        in_maps.append({
            "tbl": tbl16, "ia": ia, "ic": icx,
            "w1": w1p, "w2": w2p, "b1": b1p, "b2": b2p,
        })
    return in_maps


_NC_CACHE = {}


def get_nc():
    if "nc" not in _NC_CACHE:
        _NC_CACHE["nc"] = build_nc()
    return _NC_CACHE["nc"]


def kernel(anchor_h, candidate_h, doc_embed, W1, b1, W2, b2):
    nc = get_nc()
    in_maps = prep_inputs(anchor_h, candidate_h, doc_embed, W1, b1, W2, b2)
    res = run_bass_kernel_spmd(nc, in_maps, core_ids=list(range(CORES)))
    outs = [res.results[c]["out"] for c in range(CORES)]
    return np.concatenate(outs).reshape(BATCH, 1).astype(np.float32)



# revision 8
# speedup vs baseline: 1.1358x; 1.1358x over previous
"""Trainium2 Bass kernel for nn_Discrimitor (embedding_lookup two-tower MLP).

Strategy (8 NeuronCores, data-parallel over the batch):
  - Replicate the 1M x 100 f32 embedding table, host-cast to fp16, rows
    padded to 128 elements (256B).
  - The workload is SWDGE-issue-bound: each gpsimd indirect DMA call can
    gather only 128 rows (one offset per partition), so the baseline's
    1024 calls/core dominate wall time. To cut call count, the candidate
    (c) side uses the InstDMAGatherAnt primitive (dma_gather,
    transpose=True, single_packet=False): ONE call gathers up to 2304
    rows AND lands them embed-major (pre-transposed). int16 gather
    indices only span 32768 rows, so pairs are sorted on host by
    c-window (31 windows of 32768 rows); each window is one dma_gather
    call per core with static capacity 2304 columns (trailing -1 indices
    are skipped by the hardware; per-call true count arrives at runtime
    via a register loaded from an input tensor).
  - The anchor (a) side keeps int32 indirect gathers (128 rows/call) in
    the same sorted order; padding slots carry an out-of-bounds index and
    are silently skipped via bounds_check.
  - Pairs that overflow a window's 2304 capacity (none for uniform data;
    safety only) go to a 384-column overflow segment gathered with
    indirect calls on both sides.
  - Compute per 384-column tile: 3 PE transposes flip gathered a-rows to
    embed-major, DVE forms a*c, 3 accumulating fp16 matmuls compute
    hidden @ W1 into PSUM [64,384], ACT applies relu+b1, one matmul with
    W2 gives logits, +b2, staged and DMA'd out. The c side needs no
    transposes at all (dma_gather already delivered embed-major).
  - Host unpermutes the per-core padded logits back to batch order.

kernel() takes FULL unsharded inputs, returns the FULL [524288, 1] f32
output.
"""

import sys

for _p in ("/opt/trn_rl_repo",):
    if _p not in sys.path:
        sys.path.insert(0, _p)

import numpy as np

import concourse.bacc as bacc
import concourse.tile as tile
from concourse import bass, mybir
from concourse.bass_utils import run_bass_kernel_spmd
from concourse.masks import make_identity

# ---- problem constants (hardcoded per contract) ----
DOC_SIZE = 1_000_000
EMBED = 100
DP = 128            # padded row length (fp16 -> 256B rows)
H = 64
BATCH = 524288
CORES = 8
BC = BATCH // CORES         # 65536 batch elements per core

# ---- sharding / gather layout ----
WIN = 32768                 # rows addressable by one int16 dma_gather call
NWIN = (DOC_SIZE + WIN - 1) // WIN       # 31 windows
CAP = 2304                  # per-window column capacity (mean 2114, max seen 2267)
OVF = 384                   # overflow segment columns (both sides indirect)
COLS = NWIN * CAP + OVF     # 71808 padded columns per core
TILE = 384                  # compute tile width; CAP % TILE == 0
OOB_IDX = 1_500_000         # > DOC_SIZE-1 -> descriptor skipped

F16 = mybir.dt.float16
F32 = mybir.dt.float32
I32 = mybir.dt.int32
I16 = mybir.dt.int16
AF = mybir.ActivationFunctionType

_QNAMES = ["qPoolDynamic", "qPoolDynamic1", "qPoolDynamic2", "qPoolDynamic3"]


def build_nc(reps=1):
    nc = bacc.Bacc("TRN2", target_bir_lowering=False, num_swdge_queues=4)

    tbl = nc.dram_tensor("tbl", [DOC_SIZE, DP], F16, kind="ExternalInput")
    ia = nc.dram_tensor("ia", [128, COLS // 128], I32, kind="ExternalInput")
    ic16 = nc.dram_tensor("ic16", [128, NWIN * (CAP // 16)], I16,
                          kind="ExternalInput")
    icv = nc.dram_tensor("icv", [128, OVF // 128], I32, kind="ExternalInput")
    nw = nc.dram_tensor("nw", [1, NWIN], I32, kind="ExternalInput")
    w1 = nc.dram_tensor("w1", [DP, 3 * H], F16, kind="ExternalInput")
    w2 = nc.dram_tensor("w2", [H, 1], F16, kind="ExternalInput")
    b1 = nc.dram_tensor("b1", [H, 1], F32, kind="ExternalInput")
    b2 = nc.dram_tensor("b2", [1, 1], F32, kind="ExternalInput")
    out = nc.dram_tensor("out", [COLS], F32, kind="ExternalOutput")

    with tile.TileContext(nc) as tc:
        with (
            tc.tile_pool(name="singles", bufs=1) as singles,
            tc.tile_pool(name="graw", bufs=3) as graw,
            tc.tile_pool(name="tsb", bufs=4) as tsb,
            tc.tile_pool(name="h1sb", bufs=2) as h1sb,
            tc.tile_pool(name="stage", bufs=2) as stagep,
            tc.tile_pool(name="ps_t", bufs=4, space="PSUM") as ps_t,
            tc.tile_pool(name="ps_h1", bufs=2, space="PSUM") as ps_h1,
            tc.tile_pool(name="ps_lg", bufs=2, space="PSUM") as ps_lg,
        ):
            # constants / weights / indices -> SBUF once
            w1_sb = singles.tile([DP, 3 * H], F16)
            nc.sync.dma_start(out=w1_sb[:], in_=w1[:])
            w2_sb = singles.tile([H, 1], F16)
            nc.sync.dma_start(out=w2_sb[:], in_=w2[:])
            b1_sb = singles.tile([H, 1], F32)
            nc.sync.dma_start(out=b1_sb[:], in_=b1[:])
            b2_sb = singles.tile([1, 1], F32)
            nc.sync.dma_start(out=b2_sb[:], in_=b2[:])
            ident = singles.tile([128, 128], F16)
            make_identity(nc, ident[:])
            ia_sb = singles.tile([128, COLS // 128], I32)
            nc.sync.dma_start(out=ia_sb[:], in_=ia[:])
            ic16_sb = singles.tile([128, NWIN * (CAP // 16)], I16)
            nc.sync.dma_start(out=ic16_sb[:], in_=ic16[:])
            icv_sb = singles.tile([128, OVF // 128], I32)
            nc.sync.dma_start(out=icv_sb[:], in_=icv[:])
            nw_sb = singles.tile([1, NWIN], I32)
            nc.sync.dma_start(out=nw_sb[:], in_=nw[:])

            # per-window true-count registers (loaded once; reps loop reuses)
            nregs = []
            for w in range(NWIN):
                r = nc.gpsimd.alloc_register(f"nw{w}")
                nc.gpsimd.reg_load(r, nw_sb[0:1, w : w + 1])
                nregs.append(r)

            def compute_tile(aT_src_raw, cT_bucket, col0, width, stage,
                             stage_off, c_needs_transpose=False,
                             c_src_raw=None):
                """One compute tile of `width` columns.

                aT_src_raw: batch-major gathered a rows [128, >=col0+width]
                cT_bucket: embed-major c tile (or None if c_needs_transpose)
                """
                nblk = width // 128
                aT = tsb.tile([128, TILE], F16, tag="aT")
                for u in range(nblk):
                    k = col0 + u * 128
                    tp = ps_t.tile([128, 128], F16, tag="pst")
                    nc.tensor.transpose(tp[:], aT_src_raw[:, k : k + 128],
                                        ident[:])
                    nc.vector.tensor_copy(out=aT[:, u * 128 : (u + 1) * 128],
                                          in_=tp[:])
                if c_needs_transpose:
                    cT = tsb.tile([128, TILE], F16, tag="cT")
                    for u in range(nblk):
                        k = col0 + u * 128
                        tp = ps_t.tile([128, 128], F16, tag="pst")
                        nc.tensor.transpose(tp[:], c_src_raw[:, k : k + 128],
                                            ident[:])
                        nc.scalar.activation(
                            out=cT[:, u * 128 : (u + 1) * 128], in_=tp[:],
                            func=AF.Copy)
                    cT_ap = cT[:, 0:width]
                else:
                    cT_ap = cT_bucket[:, col0 : col0 + width]

                acT = tsb.tile([128, TILE], F16, tag="acT")
                nc.vector.tensor_mul(acT[:, 0:width], aT[:, 0:width], cT_ap)

                h1p = ps_h1.tile([H, TILE], F32, tag="h1p")
                nc.tensor.matmul(h1p[:, 0:width], w1_sb[:, 0:H],
                                 aT[:, 0:width], start=True, stop=False)
                nc.tensor.matmul(h1p[:, 0:width], w1_sb[:, H : 2 * H],
                                 cT_ap, start=False, stop=False)
                nc.tensor.matmul(h1p[:, 0:width], w1_sb[:, 2 * H : 3 * H],
                                 acT[:, 0:width], start=False, stop=True)
                h1s = h1sb.tile([H, TILE], F16, tag="h1s")
                nc.scalar.activation(out=h1s[:, 0:width], in_=h1p[:, 0:width],
                                     func=AF.Relu, bias=b1_sb[:], scale=1.0)
                lgp = ps_lg.tile([1, TILE], F32, tag="lgp")
                nc.tensor.matmul(lgp[:, 0:width], w2_sb[:], h1s[:, 0:width],
                                 start=True, stop=True)
                dst = stage[0:1, stage_off : stage_off + width]
                nc.vector.tensor_add(out=dst, in0=lgp[:, 0:width],
                                     in1=b2_sb[:].to_broadcast([1, width]))

            rep_cm = tc.For_i(0, reps) if reps > 1 else None
            if rep_cm is not None:
                rep_cm.__enter__()

            for w in range(NWIN):
                wrows = min(WIN, DOC_SIZE - w * WIN)
                # ---- c side: one dma_gather for the whole window bucket ----
                cT_b = graw.tile([128, CAP], F16, tag="cTb")
                nc.gpsimd.dma_gather(
                    out_ap=cT_b[:].rearrange("p (o n) -> p o n", o=1),
                    in_ap=tbl[w * WIN : w * WIN + wrows, :],
                    idxs_ap=ic16_sb[:, w * (CAP // 16) : (w + 1) * (CAP // 16)],
                    num_idxs=CAP,
                    num_idxs_reg=nregs[w],
                    elem_size=DP,
                    transpose=True,
                    single_packet=False,
                    queue_num=0,
                )
                # ---- a side: indirect gathers, 128 rows per call ----
                a_raw = graw.tile([128, CAP], F16, tag="araw")
                for j in range(CAP // 128):
                    jj = w * (CAP // 128) + j
                    inst = nc.gpsimd.indirect_dma_start(
                        out=a_raw[:, j * DP : (j + 1) * DP],
                        out_offset=None,
                        in_=tbl[:],
                        in_offset=bass.IndirectOffsetOnAxis(
                            ap=ia_sb[:, jj : jj + 1], axis=0),
                        bounds_check=DOC_SIZE - 1,
                        oob_is_err=False,
                    )
                    inst.ins.queue = _QNAMES[j % 4]

                stage = stagep.tile([1, CAP], F32, tag="stg")
                for t in range(CAP // TILE):
                    compute_tile(a_raw, cT_b, t * TILE, TILE, stage, t * TILE)
                nc.sync.dma_start(
                    out=out[w * CAP : (w + 1) * CAP].rearrange(
                        "(o n) -> o n", o=1),
                    in_=stage[:],
                )

            # ---- overflow segment: both sides indirect ----
            a_ov = graw.tile([128, OVF], F16, tag="aov")
            c_ov = graw.tile([128, OVF], F16, tag="cov")
            for j in range(OVF // 128):
                jj = NWIN * (CAP // 128) + j
                inst = nc.gpsimd.indirect_dma_start(
                    out=a_ov[:, j * DP : (j + 1) * DP],
                    out_offset=None, in_=tbl[:],
                    in_offset=bass.IndirectOffsetOnAxis(
                        ap=ia_sb[:, jj : jj + 1], axis=0),
                    bounds_check=DOC_SIZE - 1, oob_is_err=False,
                )
                inst.ins.queue = _QNAMES[j % 4]
                inst = nc.gpsimd.indirect_dma_start(
                    out=c_ov[:, j * DP : (j + 1) * DP],
                    out_offset=None, in_=tbl[:],
                    in_offset=bass.IndirectOffsetOnAxis(
                        ap=icv_sb[:, j : j + 1], axis=0),
                    bounds_check=DOC_SIZE - 1, oob_is_err=False,
                )
                inst.ins.queue = _QNAMES[(j + 2) % 4]
            stage_ov = stagep.tile([1, OVF], F32, tag="stgov")
            compute_tile(a_ov, None, 0, OVF, stage_ov, 0,
                         c_needs_transpose=True, c_src_raw=c_ov)
            nc.sync.dma_start(
                out=out[NWIN * CAP : COLS].rearrange("(o n) -> o n", o=1),
                in_=stage_ov[:],
            )

            if rep_cm is not None:
                rep_cm.__exit__(None, None, None)

    nc.compile()
    return nc


def prep_inputs(anchor_h, candidate_h, doc_embed, W1, b1, W2, b2):
    """Host-side packing of full inputs into per-core in_maps + unpermute
    metadata (per-core padded position of each original pair)."""
    tbl16 = np.zeros((DOC_SIZE, DP), np.float16)
    tbl16[:, :EMBED] = np.asarray(doc_embed, np.float32)

    W1 = np.asarray(W1, np.float32)
    w1p = np.zeros((DP, 3 * H), np.float16)
    w1p[:EMBED, 0:H] = W1[0:EMBED]
    w1p[:EMBED, H : 2 * H] = W1[EMBED : 2 * EMBED]
    w1p[:EMBED, 2 * H : 3 * H] = W1[2 * EMBED : 3 * EMBED]

    w2p = np.asarray(W2, np.float32).astype(np.float16).reshape(H, 1)
    b1p = np.asarray(b1, np.float32).reshape(H, 1)
    b2p = np.asarray(b2, np.float32).reshape(1, 1)

    a_all = np.asarray(anchor_h).astype(np.int64)
    c_all = np.asarray(candidate_h).astype(np.int64)

    in_maps, positions = [], []
    for k in range(CORES):
        sl = slice(k * BC, (k + 1) * BC)
        a = a_all[sl]
        c = c_all[sl]
        wc = c // WIN
        order = np.argsort(wc, kind="stable")

        a_pad = np.full(COLS, OOB_IDX, np.int64)
        c16_pad = np.full(NWIN * CAP, -1, np.int16)
        cov_pad = np.full(OVF, OOB_IDX, np.int64)
        nw_arr = np.ones(NWIN, np.int32)
        pos = np.empty(BC, np.int64)

        counts = np.bincount(wc, minlength=NWIN)
        start = 0
        ovf_fill = 0
        for w in range(NWIN):
            n = int(counts[w])
            grp = order[start : start + n]
            start += n
            take = grp[:CAP]
            rest = grp[CAP:]
            m = len(take)
            pos[take] = w * CAP + np.arange(m)
            a_pad[w * CAP : w * CAP + m] = a[take]
            c16_pad[w * CAP : w * CAP + m] = (c[take] - w * WIN).astype(
                np.int16)
            if m == 0:
                c16_pad[w * CAP] = 0  # keep >=1 valid idx per call
            nw_arr[w] = max(m, 1)
            for r in rest:
                assert ovf_fill < OVF, "overflow segment exhausted"
                pos[r] = NWIN * CAP + ovf_fill
                a_pad[NWIN * CAP + ovf_fill] = a[r]
                cov_pad[ovf_fill] = c[r]
                ovf_fill += 1

        # a-side offsets layout: [p, blk] = padded column blk*128+p
        ia32 = a_pad.reshape(COLS // 128, 128).T.astype(np.int32).copy()
        icv32 = cov_pad.reshape(OVF // 128, 128).T.astype(np.int32).copy()
        # c-side int16, wrap-16: index i of bucket w at [i%16, w*144 + i//16]
        ic16 = np.zeros((128, NWIN * (CAP // 16)), np.int16)
        blk = c16_pad.reshape(NWIN, CAP // 16, 16)
        for rep in range(8):
            ic16[rep * 16 : (rep + 1) * 16, :] = (
                blk.transpose(2, 0, 1).reshape(16, -1))
        in_maps.append({
            "tbl": tbl16, "ia": ia32, "ic16": ic16, "icv": icv32,
            "nw": nw_arr.reshape(1, NWIN),
            "w1": w1p, "w2": w2p, "b1": b1p, "b2": b2p,
        })
        positions.append(pos)
    return in_maps, positions


_NC_CACHE = {}


def get_nc():
    if "nc" not in _NC_CACHE:
        _NC_CACHE["nc"] = build_nc()
    return _NC_CACHE["nc"]


def kernel(anchor_h, candidate_h, doc_embed, W1, b1, W2, b2):
    nc = get_nc()
    in_maps, positions = prep_inputs(anchor_h, candidate_h, doc_embed,
                                     W1, b1, W2, b2)
    res = run_bass_kernel_spmd(nc, in_maps, core_ids=list(range(CORES)))
    full = np.empty(BATCH, np.float32)
    for k in range(CORES):
        padded = res.results[k]["out"]
        full[k * BC : (k + 1) * BC] = padded[positions[k]]
    return full.reshape(BATCH, 1).astype(np.float32)
